# revision 42
# baseline (speedup 1.0000x reference)
"""Trainium2 Bass kernel for nn_ArmRGBReg (retrieval-KNN), SPMD on 8 NeuronCores.

Sharding: the 8000 lower-arm rows are x-sorted on the host and split into 8
shards of 1000 (8 blocks of 125 rows each per core).  Per the sharding hint,
the host gathers mesh[upper_idx]/mesh[lower_idx] (index-only work) while
sharding, so each core receives its operands pre-packed in final layout.
Coordinates are centered per block (x at the block window center, y/z at
0.5) so the bilinear forms below are well-conditioned in fp32:
  - U4 [4, 8*384]  f32: per window slot [2x', 2y', 2z', C-|u'|^2],  C=2.25
  - V3 [3, 8*384]  f32: per window slot [1, -2x', x'^2]
  - L4 [4, 8*128]  f32: per row [lx', ly', lz', 1]
  - L3 [3, 8*128]  f32: per row [lx'^2, lx', 1]
  - rw [128, 8*3*24] bf16: window rgb (slot-major chunks of 128)
  - rl [24, 8*128] f32: lower rgb
Each block's 384-slot candidate window is the x-sorted slice of upper
vertices within the block's x-interval +-0.0101 (max seen ~360), padded with
a sentinel (U4 col [0,0,0,C], V3 col [1,-4,4]) that fails the proximity
test.

Per block the device computes (software-pipelined across blocks so every
engine stays busy):
  FRONT: TensorE negkey = 2 l'.u' + C - |u'|^2 (= C + |l'|^2 - dist^2) and
     dx2 = (lx'-ux')^2, two fp32 matmuls into PSUM; ScalarE copies negkey to
     SBUF and computes sgn = Sign(1e-4 - dx2); GpSimd multiplies
     key = negkey * sgn (x-invalid candidates go negative, below all valid).
  TOPK (DVE): L1 = 12 stride-interleaved groups of 32 -> top-8 via one max8
     each (x-sorted window + striding keeps per-group membership of the true
     top-50 under 8 w.h.p.); L2 = 7 rounds of max8 over the 96 survivors,
     pruning extracted ranks with (cur < v8) * cur between rounds; vals[49]
     is the rank-50 key.  GpSimd: mask Mm = (key >= vals[49]) in bf16.
  MID: DMA-engine transposes of Mm; TensorE neighbor-sum = rgb_win^T @ Mm^T.
  TAIL: loss = (sum/50 - rgb_lower)^2 (DVE + ScalarE Square).
Host work is layout-only: sorting/grouping indices, gathering rows by the
given indices, packing tiles, scattering per-core outputs back to [8,8000,3].
"""

import numpy as np
import ml_dtypes

import concourse.bass as bass
import concourse.bacc as bacc
import concourse.mybir as mybir
from concourse.bass_utils import run_bass_kernel_spmd
from concourse.masks import make_identity
from concourse.tile import TileContext

V = 107778
B = 8
NU = 8000
NL = 8000
K = 50
P = 128
BC = B * 3
NBLK = 8              # row blocks per core
RPB = 125             # real rows per block (8*125 = 1000)
WIN = 3 * P           # 384-slot candidate window per block
NG = 12               # L1 stride-interleaved groups (32 slots each)
NS = NG * 8           # L1 survivors (96)
CC = 2.25             # negkey constant: nk = 2l'.u' + CC - |u'|^2
F32 = mybir.dt.float32
BF16 = mybir.dt.bfloat16
Alu = mybir.AluOpType
Act = mybir.ActivationFunctionType
XMARGIN = 0.0101      # host window half-width guard


def build_graph():
    nc = bacc.Bacc()
    uv_ext = nc.declare_dram_parameter("uv", [4, 2 * NBLK * WIN], F32, isOutput=False)
    ll_ext = nc.declare_dram_parameter("ll", [4, 2 * NBLK * P], F32, isOutput=False)
    rw_ext = nc.declare_dram_parameter("rw", [P, NBLK * 3 * BC], BF16, isOutput=False)
    rl_ext = nc.declare_dram_parameter("rl", [P, NBLK * BC], F32, isOutput=False)
    out_ext = nc.declare_dram_parameter("out", [BC, NBLK * P], F32, isOutput=True)

    with TileContext(nc) as tc:
        with (
            tc.tile_pool(name="persist", bufs=1) as pp,
            tc.tile_pool(name="work", bufs=4) as wp,
            tc.tile_pool(name="psum_n", bufs=2, space="PSUM") as pn,
            tc.tile_pool(name="psum_d", bufs=2, space="PSUM") as pdk,
            tc.tile_pool(name="psum_m", bufs=2, space="PSUM") as pm,
            tc.tile_pool(name="psum_o", bufs=2, space="PSUM") as po,
        ):
            thr_t = pp.tile([P, 1], F32)
            nc.vector.memset(thr_t[:, 0:1], 1e-4)
            ident = pp.tile([P, P], F32)
            make_identity(nc, ident[:])
            ident16 = pp.tile([P, P], BF16)
            nc.vector.tensor_copy(ident16[:], ident[:])

            uv = pp.tile([4, 2 * NBLK * WIN], F32)
            nc.sync.dma_start(out=uv[:], in_=uv_ext[:])
            ll = pp.tile([4, 2 * NBLK * P], F32)
            nc.scalar.dma_start(out=ll[:], in_=ll_ext[:])
            rw = pp.tile([P, NBLK, 3, BC], BF16)
            nc.sync.dma_start(out=rw[:], in_=rw_ext[:])
            rl = pp.tile([P, NBLK, BC], F32)
            nc.sync.dma_start(out=rl[:], in_=rl_ext[:])
            out_sb = pp.tile([BC, NBLK * P], F32)

            # Warm-ups while the input DMAs land: dummy activations trigger the
            # act-table loads for Copy/Sign/Square off the critical path, and
            # dummy transposes keep the tensor engine's p-state ramp alive so
            # the first key matmuls run near full rate.
            awarm = pp.tile([P, 1], F32)
            nc.scalar.copy(out=awarm[:, 0:1], in_=thr_t[:, 0:1])
            nc.scalar.activation(out=awarm[:, 0:1], in_=thr_t[:, 0:1],
                                 func=Act.Sign, bias=thr_t[:, 0:1], scale=-1.0)
            nc.scalar.activation(out=awarm[:, 0:1], in_=thr_t[:, 0:1],
                                 func=Act.Square)
            warm = pm.tile([P, 3, P], BF16, tag="ptM")
            for _ in range(3):
                nc.tensor.transpose(out=warm[:, 0, :], in_=ident16[:],
                                    identity=ident16[:])

            tiles = [dict() for _ in range(NBLK)]

            def front(t):
                d = tiles[t]
                usl = slice(t * WIN, (t + 1) * WIN)
                vsl = slice(NBLK * WIN + t * WIN, NBLK * WIN + (t + 1) * WIN)
                lsl = slice(t * P, (t + 1) * P)
                l3sl = slice(NBLK * P + t * P, NBLK * P + (t + 1) * P)
                psN = pn.tile([P, WIN], F32, tag="psN")
                psD = pdk.tile([P, WIN], F32, tag="psD")
                nc.tensor.matmul(out=psN[:], lhsT=ll[:, lsl], rhs=uv[:, usl],
                                 start=True, stop=True)
                nc.tensor.matmul(out=psD[:], lhsT=ll[0:3, l3sl],
                                 rhs=uv[0:3, vsl], start=True, stop=True)
                pf = wp.tile([P, WIN], F32, tag="pf")
                if t == 0:
                    # DVE is idle during startup: build bin 0's keys there so
                    # the first top-k starts without the Act/Pool chain
                    nsb = wp.tile([P, WIN], F32, tag="nsb")
                    nc.vector.tensor_copy(nsb[:], psN[:])
                    nc.vector.scalar_tensor_tensor(
                        out=pf[:], in0=psD[:], scalar=1e-4, in1=nsb[:],
                        op0=Alu.is_lt, op1=Alu.mult)
                else:
                    nsb = wp.tile([P, WIN], F32, tag="nsb")
                    nc.scalar.copy(out=nsb[:], in_=psN[:])
                    sgn = wp.tile([P, WIN], F32, tag="sgn")
                    nc.scalar.activation(out=sgn[:], in_=psD[:], func=Act.Sign,
                                         bias=thr_t[:, 0:1], scale=-1.0)
                    nc.gpsimd.tensor_tensor(out=pf[:], in0=nsb[:], in1=sgn[:],
                                            op=Alu.mult)
                d["pf"] = pf

            def topk(t):
                d = tiles[t]
                pf = d["pf"]
                pfs = pf[:].rearrange("p (w s) -> p w s", s=NG)
                lvl1 = wp.tile([P, NS], F32, tag="lvl1")
                for g in range(NG):
                    nc.vector.max(out=lvl1[:, g * 8:(g + 1) * 8], in_=pfs[:, :, g])
                vals = wp.tile([P, 56], F32, tag="vals")
                cur = lvl1
                for r in range(7):
                    nc.vector.max(out=vals[:, r * 8:(r + 1) * 8], in_=cur[:])
                    if r < 6:
                        nxt = wp.tile([P, NS], F32, tag=f"cur{r % 2}")
                        nc.vector.scalar_tensor_tensor(
                            out=nxt[:], in0=cur[:],
                            scalar=vals[:, r * 8 + 7:r * 8 + 8], in1=cur[:],
                            op0=Alu.is_lt, op1=Alu.mult)
                        cur = nxt
                Mm = wp.tile([P, WIN], BF16, tag="Mm")
                if t == NBLK - 1:
                    # DVE is free after the last top-k; skip the Pool queue
                    nc.vector.tensor_scalar(out=Mm[:], in0=pf[:],
                                            scalar1=vals[:, 49:50], scalar2=None,
                                            op0=Alu.is_ge)
                else:
                    nc.gpsimd.tensor_scalar(out=Mm[:], in0=pf[:],
                                            scalar1=vals[:, 49:50], scalar2=None,
                                            op0=Alu.is_ge)
                d["Mm"] = Mm

            def mid(t):
                d = tiles[t]
                Mm = d["Mm"]
                ptM = pm.tile([P, 3, P], BF16, tag="ptM")
                for dt in range(3):
                    nc.tensor.transpose(out=ptM[:, dt, :],
                                        in_=Mm[:, dt * P:(dt + 1) * P],
                                        identity=ident16[:])
                MT = wp.tile([P, 3, P], BF16, tag="MT")
                if t == NBLK - 1:
                    # DVE is free after the last top-k; skip the Act queue
                    nc.vector.tensor_copy(MT[:], ptM[:])
                else:
                    nc.scalar.copy(out=MT[:], in_=ptM[:])
                psO = po.tile([BC, P], F32, tag="psO")
                for dt in range(3):
                    nc.tensor.matmul(out=psO[:], lhsT=rw[:, t, dt, :],
                                     rhs=MT[:, dt, :],
                                     start=(dt == 0), stop=False)
                # psO += -rgb_lower: rl holds -rgb_low^T, contract with identity
                nc.tensor.matmul(out=psO[:], lhsT=rl[:, t, :], rhs=ident[:],
                                 start=False, stop=True)
                d["psO"] = psO

            def tail(t):
                d = tiles[t]
                lsl = slice(t * P, (t + 1) * P)
                nc.scalar.activation(out=out_sb[:, lsl], in_=d["psO"][:],
                                     func=Act.Square)
                nc.sync.dma_start(out=out_ext[:, lsl], in_=out_sb[:, lsl])

            for it in range(NBLK + 3):
                if it < NBLK:
                    front(it)
                if 2 <= it < NBLK + 2:
                    topk(it - 2)
                if 2 <= it < NBLK + 2:
                    mid(it - 2)
                if it >= 3:
                    tail(it - 3)
    nc.compile()
    return nc


def kernel(mesh_neutral_pose, rgb, upper_idx, lower_idx, _trace=False):
    mesh = np.ascontiguousarray(np.asarray(mesh_neutral_pose, dtype=np.float32))
    rgb_np = np.asarray(rgb, dtype=np.float32)
    up = np.asarray(upper_idx).astype(np.int64)
    lo = np.asarray(lower_idx).astype(np.int64)
    lx = np.float64(mesh[lo, 0])
    ux = np.float64(mesh[up, 0])
    order = np.argsort(lx, kind="stable")
    uord = np.argsort(ux, kind="stable")
    up_s = up[uord]
    ux_s = ux[uord]
    # rgb in [vertex, b*3+c] layout for fast row gathers
    rgb_vc = np.ascontiguousarray(rgb_np.transpose(1, 0, 2).reshape(V, BC))

    nc = build_graph()
    in_maps = []
    slotmaps = []
    for c in range(8):
        crows = order[c * NL // 8:(c + 1) * NL // 8]
        uv = np.zeros((4, 2 * NBLK * WIN), np.float32)
        ll = np.zeros((4, 2 * NBLK * P), np.float32)
        u4 = uv[:, :NBLK * WIN]
        v3 = uv[0:3, NBLK * WIN:]
        l4 = ll[:, :NBLK * P]
        l3 = ll[0:3, NBLK * P:]
        rw = np.zeros((P, NBLK, 3, BC), ml_dtypes.bfloat16)
        rl = np.zeros((P, NBLK, BC), np.float32)
        # sentinel window slot: u'=(0,0,0) keeps negkey=CC>0; V3 pad fails dx2
        u4[3, :] = CC
        v3[:, :] = np.array([1.0, -4.0, 4.0], np.float32)[:, None]
        smap = np.empty((NBLK, P), np.int64)
        smap.fill(-1)
        for k in range(NBLK):
            blk = crows[k * RPB:(k + 1) * RPB]
            smap[k, :len(blk)] = blk
            a, b = lx[blk].min(), lx[blk].max()
            x0 = np.float32((a + b) * 0.5)
            mb = mesh[lo[blk]] - np.array([x0, 0.5, 0.5], np.float32)
            sl = slice(k * P, k * P + len(blk))
            l4[0:3, sl] = mb.T
            l4[3, sl] = 1.0
            l3[0, sl] = mb[:, 0] * mb[:, 0]
            l3[1, sl] = mb[:, 0]
            l3[2, sl] = 1.0
            rl[:len(blk), k, :] = -rgb_vc[lo[blk]]
            i0 = np.searchsorted(ux_s, a - XMARGIN, side="left")
            i1 = np.searchsorted(ux_s, b + XMARGIN, side="right")
            seg = up_s[i0:i1]
            assert len(seg) <= WIN, f"window overflow {len(seg)}"
            cu = mesh[seg] - np.array([x0, 0.5, 0.5], np.float32)
            wsl = slice(k * WIN, k * WIN + len(seg))
            u4[0:3, wsl] = 2.0 * cu.T
            u4[3, wsl] = CC - (cu * cu).sum(1)
            v3[0, wsl] = 1.0
            v3[1, wsl] = -2.0 * cu[:, 0]
            v3[2, wsl] = cu[:, 0] * cu[:, 0]
            rwk = np.zeros((WIN, BC), np.float32)
            rwk[:len(seg)] = rgb_vc[seg] * np.float32(1.0 / K)
            rw[:, k, :, :] = rwk.reshape(3, P, BC).transpose(1, 0, 2)
        slotmaps.append(smap)
        in_maps.append({
            "uv": uv, "ll": ll,
            "rw": rw.reshape(P, NBLK * 3 * BC), "rl": rl.reshape(P, NBLK * BC),
        })
    res = run_bass_kernel_spmd(nc, in_maps, core_ids=list(range(8)), trace=_trace)
    out = np.empty((B, NL, 3), np.float32)
    for c in range(8):
        o = np.asarray(res.results[c]["out"]).reshape(B, 3, NBLK, P)
        smap = slotmaps[c]
        for k in range(NBLK):
            valid = smap[k] >= 0
            rows = smap[k][valid]
            out[:, rows, :] = o[:, :, k, valid].transpose(0, 2, 1)
    if _trace:
        return out, res
    return out
